# revision 4
# baseline (speedup 1.0000x reference)
"""TRN2 Bass kernel for nn_DecompModel (GNN edge scorer), 8-core SPMD.

kernel(**inputs) takes the FULL inputs and returns the full 5-tuple
(scores, y_st, new_mask, causal_w, spu_w), computed on 8 NeuronCores.

Strategy (see block comments): edges are partitioned by source node across
cores; per core, nodes are degree-sorted into blocks of 128 partitions with
per-block slot columns, so the segment softmax is a free-dim reduction.
Edge MLP runs on the PE as a single plain fp32 matmul per slot column
(per-instruction cost dominates on this platform, and fp32 matmul is the
cheapest instruction); the W2 contraction is fused into ACT Relu accum
with |w2| folded into W1 (positive/negative column groups). The Bernoulli
threshold uses the log-free  e*u + (e-s)*(1-u) > 0  with polynomial exp.
"""
import numpy as np
import concourse.bass as bass
import concourse.bacc as bacc
import concourse.mybir as mybir
import concourse.tile as tile
from concourse.bass_utils import run_bass_kernel_spmd

F32 = mybir.dt.float32
F32R = mybir.dt.float32r
I32 = mybir.dt.int32
I16 = mybir.dt.int16
ALU = mybir.AluOpType
ACTF = mybir.ActivationFunctionType

N_CORES = 8
EMB = 128
HID = 512

_EXP_C = [1.0, 1.0, 1 / 2, 1 / 6, 1 / 24, 1 / 120, 1 / 720, 1 / 5040]
_LOG2E = 1.4426950408889634
_LN2_HI = np.float32(0.6931471824645996)
_LN2_LO = np.float32(-1.904654323148236e-09)
_MAGIC = np.float32(12582912.0)


def _pack_inputs(h, W1, b1, W2, b2, u, row, col, edge_mask, hierarchy):
    N = h.shape[0]
    E = row.shape[0]
    deg = np.bincount(row, minlength=N)
    cum = np.cumsum(deg)
    bounds = [0]
    for c in range(1, N_CORES):
        bounds.append(int(np.searchsorted(cum, E * c / N_CORES)))
    bounds.append(N)

    order = np.argsort(row, kind="stable")
    starts = np.concatenate([[0], cum])

    maxnodes = max(b - a for a, b in zip(bounds[:-1], bounds[1:]))
    NB = (maxnodes + 127) // 128

    core_nodes = []
    Ds = np.zeros((N_CORES, NB), np.int64)
    for c in range(N_CORES):
        a, b = bounds[c], bounds[c + 1]
        nodes = np.arange(a, b)
        srt = nodes[np.argsort(-deg[a:b], kind="stable")]
        srt = np.concatenate([srt, np.zeros(NB * 128 - len(srt), np.int64)])
        dg = deg[srt].copy()
        dg[b - a:] = 0
        core_nodes.append((srt, dg))
        Ds[c] = dg.reshape(NB, 128).max(1)
    D = np.maximum(Ds.max(0).astype(int), 1)
    SUMD = int(D.sum())
    offs = np.concatenate([[0], np.cumsum(D)]).astype(int)

    aw2 = np.abs(W2[:, 0])
    sgn_neg = W2[:, 0] < 0
    perm = np.argsort(sgn_neg, kind="stable")
    PP = int((~sgn_neg).sum())
    W1s = (W1 * aw2[None, :])[:, perm]
    b1s = (b1 * aw2)[perm]
    assert np.abs(b1s).max() == 0.0, "nonzero b1 not wired up"
    Wtop = np.ascontiguousarray(W1s[:EMB])
    Wbot = np.ascontiguousarray(W1s[EMB:])

    def wrap16(idxs):
        n = len(idxs)
        w = np.zeros((16, n // 16), np.int16)
        ii = np.arange(n)
        w[ii % 16, ii // 16] = idxs
        return np.tile(w, (8, 1))

    in_maps = []
    origs = []
    for c in range(N_CORES):
        srt, dg = core_nodes[c]
        colgrid = np.zeros((128, SUMD), np.int64)
        ugrid = np.full((128, SUMD), 0.5, np.float32)
        valid = np.zeros((128, SUMD), np.float32)
        emi = np.full((128, SUMD), -1, np.int32)
        origpos = np.full((128, SUMD), -1, np.int64)
        for b in range(NB):
            for p in range(128):
                node = srt[b * 128 + p]
                d = dg[b * 128 + p]
                if d == 0:
                    continue
                eids = order[starts[node]:starts[node] + d]
                sl = slice(offs[b], offs[b] + d)
                colgrid[p, sl] = col[eids]
                ugrid[p, sl] = u[eids]
                valid[p, sl] = 1.0
                emi[p, sl] = edge_mask[eids]
                origpos[p, sl] = eids
        nodeidx = wrap16(srt.astype(np.int16))
        colw = np.concatenate(
            [wrap16(colgrid[:, offs[b]:offs[b + 1]].T.ravel().astype(np.int16))
             for b in range(NB)], axis=1)
        in_maps.append({
            "h": np.ascontiguousarray(h),
            "wtop": Wtop, "wbot": Wbot,
            "nodeidx": nodeidx, "colidx": colw,
            "ugrid": ugrid, "valid": valid,
            "emaski": emi,
            "empos": (emi > 0).astype(np.float32),
            "emneg1": (emi == -1).astype(np.float32),
            "ident": np.eye(128, dtype=np.float32),
        })
        origs.append(origpos)
    meta = dict(NB=NB, D=D, SUMD=SUMD, offs=offs, PP=PP, origpos=origs,
                E=E, N=N, hier1=int(hierarchy) + 1, b2=float(np.ravel(b2)[0]))
    return meta, in_maps


def _build_program(meta, repeats=1):
    NB, D, SUMD, offs, PP = (meta["NB"], meta["D"], meta["SUMD"],
                             meta["offs"], meta["PP"])
    N = meta["N"]
    nc = bacc.Bacc("TRN2", target_bir_lowering=False, debug=False,
                   num_devices=N_CORES)
    dt_ = nc.dram_tensor
    h_d = dt_("h", [N, EMB], F32, kind="ExternalInput").ap()
    wtop_d = dt_("wtop", [EMB, HID], F32, kind="ExternalInput").ap()
    wbot_d = dt_("wbot", [EMB, HID], F32, kind="ExternalInput").ap()
    nodeidx_d = dt_("nodeidx", [128, NB * 8], I16, kind="ExternalInput").ap()
    colidx_d = dt_("colidx", [128, 8 * SUMD], I16, kind="ExternalInput").ap()
    ugrid_d = dt_("ugrid", [128, SUMD], F32, kind="ExternalInput").ap()
    valid_d = dt_("valid", [128, SUMD], F32, kind="ExternalInput").ap()
    emaski_d = dt_("emaski", [128, SUMD], I32, kind="ExternalInput").ap()
    empos_d = dt_("empos", [128, SUMD], F32, kind="ExternalInput").ap()
    emneg1_d = dt_("emneg1", [128, SUMD], F32, kind="ExternalInput").ap()
    ident_d = dt_("ident", [128, 128], F32, kind="ExternalInput").ap()

    scores_o = dt_("scores", [128, SUMD], F32, kind="ExternalOutput").ap()
    yst_o = dt_("yst", [128, SUMD], F32, kind="ExternalOutput").ap()
    nm_o = dt_("nm", [128, SUMD], I32, kind="ExternalOutput").ap()
    cw_o = dt_("cw", [128, SUMD], F32, kind="ExternalOutput").ap()
    sw_o = dt_("sw", [128, SUMD], F32, kind="ExternalOutput").ap()

    with tile.TileContext(nc) as tc:
        with (
            tc.tile_pool(name="const", bufs=1) as cpool,
            tc.tile_pool(name="grids", bufs=1) as gpool,
            tc.tile_pool(name="gath", bufs=3) as gath,
            tc.tile_pool(name="work", bufs=10) as work,
            tc.tile_pool(name="scr", bufs=8) as scrp,
            tc.tile_pool(name="ps_tr", bufs=2, space="PSUM") as ps_tr,
            tc.tile_pool(name="ps_hid", bufs=6, space="PSUM") as ps_hid,
        ):
            wtop_f = cpool.tile([EMB, HID], F32, tag="wtopf")
            nc.gpsimd.dma_start(out=wtop_f[:], in_=wtop_d[:])
            wbot_f = cpool.tile([EMB, HID], F32, tag="wbotf")
            nc.gpsimd.dma_start(out=wbot_f[:], in_=wbot_d[:])
            ident = cpool.tile([128, 128], F32, tag="ident")
            nc.gpsimd.dma_start(out=ident[:], in_=ident_d[:])

            nodeidx = cpool.tile([128, NB * 8], I16, tag="nodeidx")
            nc.gpsimd.dma_start(out=nodeidx[:], in_=nodeidx_d[:])
            colidx = cpool.tile([128, 8 * SUMD], I16, tag="colidx")
            nc.gpsimd.dma_start(out=colidx[:], in_=colidx_d[:])
            ugrid = gpool.tile([128, SUMD], F32, tag="ugrid")
            nc.gpsimd.dma_start(out=ugrid[:], in_=ugrid_d[:])
            validg = gpool.tile([128, SUMD], F32, tag="valid")
            nc.gpsimd.dma_start(out=validg[:], in_=valid_d[:])
            emaski = gpool.tile([128, SUMD], I32, tag="emaski")
            nc.gpsimd.dma_start(out=emaski[:], in_=emaski_d[:])
            empos = gpool.tile([128, SUMD], F32, tag="empos")
            nc.gpsimd.dma_start(out=empos[:], in_=empos_d[:])
            emneg1 = gpool.tile([128, SUMD], F32, tag="emneg1")
            nc.gpsimd.dma_start(out=emneg1[:], in_=emneg1_d[:])

            ones = gpool.tile([128, SUMD], F32, tag="ones")
            nc.vector.memset(ones[:], 1.0)
            hierc = gpool.tile([128, SUMD], I32, tag="hierc")
            nc.vector.memset(hierc[:], meta["hier1"])

            a_sb_all = gpool.tile([128, NB * HID], F32, tag="asball")
            accp = gpool.tile([128, SUMD], F32, tag="accp")
            accn = gpool.tile([128, SUMD], F32, tag="accn")
            scoresg = gpool.tile([128, SUMD], F32, tag="scoresg")
            ystg = gpool.tile([128, SUMD], F32, tag="ystg")

            for rep in range(repeats):
              # --- prelude: row-side a_r for all blocks, phased ---
              for b0 in range(0, NB, 4):
                  bn = min(4, NB - b0)
                  rgs = []
                  for t in range(bn):
                      hrow_g = gath.tile([128, 1, 128], F32, tag="hrowg")
                      nc.gpsimd.dma_gather(
                          out_ap=hrow_g[:], in_ap=h_d[:],
                          idxs_ap=nodeidx[:, (b0 + t) * 8:(b0 + t + 1) * 8],
                          num_idxs=128, num_idxs_reg=128, elem_size=EMB)
                      rgs.append(hrow_g)
                  trp = ps_tr.tile([128, 512], F32, tag="tr")
                  for t in range(bn):
                      nc.tensor.transpose(trp[:, t * 128:(t + 1) * 128],
                                          rgs[t][:, 0, :], ident[:])
                  hr1 = work.tile([128, 512], F32, tag="hc")
                  nc.vector.tensor_copy(hr1[:, :bn * 128], trp[:, :bn * 128])
                  a_pss = []
                  for t in range(bn):
                      a_ps = ps_hid.tile([128, HID], F32, tag="hid")
                      nc.tensor.matmul(a_ps[:], hr1[:, t * 128:(t + 1) * 128],
                                       wtop_f[:], start=True, stop=True)
                      a_pss.append(a_ps)
                  for t in range(bn):
                      bb = b0 + t
                      nc.vector.tensor_copy(a_sb_all[:, bb * HID:(bb + 1) * HID],
                                            a_pss[t][:])
              for b in range(NB):
                  Db = int(D[b])
                  off = int(offs[b])
                  a_sb = a_sb_all[:, b * HID:(b + 1) * HID]

                  hcol_g = gath.tile([128, Db, 128], F32, tag="hcolg")
                  CH = 8  # slot-columns per dma_gather (1024 idxs)
                  for j0 in range(0, Db, CH):
                      jn = min(CH, Db - j0)
                      nc.gpsimd.dma_gather(
                          out_ap=hcol_g[:, j0:j0 + jn, :],
                          in_ap=h_d[:],
                          idxs_ap=colidx[:, 8 * (off + j0):8 * (off + j0 + jn)],
                          num_idxs=128 * jn, num_idxs_reg=128 * jn,
                          elem_size=EMB)

                  PC = 8  # phase chunk
                  for j0 in range(0, Db, PC):
                      jn = min(PC, Db - j0)
                      ngrp = (jn + 3) // 4
                      trcs = []
                      for g in range(ngrp):
                          trc = ps_tr.tile([128, 512], F32, tag="tr")
                          trcs.append(trc)
                      for t in range(jn):
                          nc.tensor.transpose(
                              trcs[t // 4][:, (t % 4) * 128:(t % 4 + 1) * 128],
                              hcol_g[:, j0 + t, :], ident[:])
                      hcs = []
                      for g in range(ngrp):
                          hc = work.tile([128, 512], F32, tag="hc")
                          nc.vector.tensor_copy(hc[:], trcs[g][:])
                          hcs.append(hc)
                      hid_pss = []
                      for t in range(jn):
                          hid_ps = ps_hid.tile([128, HID], F32, tag="hid")
                          nc.tensor.matmul(
                              hid_ps[:],
                              hcs[t // 4][:, (t % 4) * 128:(t % 4 + 1) * 128],
                              wbot_f[:], start=True, stop=True)
                          hid_pss.append(hid_ps)
                      for t in range(jn):
                          jj = off + j0 + t
                          hidf = work.tile([128, HID], F32, tag="hidf")
                          nc.vector.tensor_add(hidf[:], hid_pss[t][:], a_sb)
                          scr = scrp.tile([128, HID], F32, tag="scr")
                          nc.scalar.activation(scr[:, :PP], hidf[:, :PP], ACTF.Relu,
                                               accum_out=accp[:, jj:jj + 1])
                          nc.scalar.activation(scr[:, PP:], hidf[:, PP:], ACTF.Relu,
                                               accum_out=accn[:, jj:jj + 1])

              # ---- global post chain over [128, SUMD] ----
              sc = scoresg[:]
              nc.vector.tensor_sub(sc, accp[:], accn[:])
              if meta["b2"] != 0.0:
                  nc.vector.tensor_scalar_add(sc, sc, meta["b2"])
              t_kf = gpool.tile([128, SUMD], F32, tag="p2a")
              nc.vector.tensor_scalar_mul(t_kf[:], sc, float(_LOG2E))
              t_m = gpool.tile([128, SUMD], F32, tag="p2b")
              nc.vector.tensor_scalar_add(t_m[:], t_kf[:], float(_MAGIC))
              t_kr = gpool.tile([128, SUMD], F32, tag="p2c")
              nc.vector.tensor_scalar_sub(t_kr[:], t_m[:], float(_MAGIC))
              t_f = gpool.tile([128, SUMD], F32, tag="p2d")
              nc.vector.scalar_tensor_tensor(
                  out=t_f[:], in0=t_kr[:], scalar=-float(_LN2_HI),
                  in1=sc, op0=ALU.mult, op1=ALU.add)
              nc.vector.scalar_tensor_tensor(
                  out=t_f[:], in0=t_kr[:], scalar=-float(_LN2_LO),
                  in1=t_f[:], op0=ALU.mult, op1=ALU.add)
              t_acc = gpool.tile([128, SUMD], F32, tag="p2e")
              nc.vector.memset(t_acc[:], _EXP_C[-1])
              for kcoef in reversed(_EXP_C[:-1]):
                  nc.vector.scalar_tensor_tensor(
                      out=t_acc[:], in0=t_acc[:], scalar=0.0,
                      in1=t_f[:], op0=ALU.add, op1=ALU.mult)
                  nc.vector.tensor_scalar_add(t_acc[:], t_acc[:], float(kcoef))
              t_mi = gpool.tile([128, SUMD], I32, tag="p2l")
              nc.vector.tensor_copy(t_mi[:], t_m[:])
              t_bits = gpool.tile([128, SUMD], I32, tag="p2f")
              nc.vector.tensor_scalar(
                  out=t_bits[:], in0=t_mi[:],
                  scalar1=(127 - 12582912), scalar2=(1 << 23),
                  op0=ALU.add, op1=ALU.mult)
              e_t = gpool.tile([128, SUMD], F32, tag="p2g")
              nc.vector.tensor_mul(e_t[:], t_acc[:], t_bits[:].bitcast(F32))
              nc.vector.tensor_mul(e_t[:], e_t[:], validg[:])
              g_t = gpool.tile([128, SUMD], F32, tag="p2h")
              for b in range(NB):
                  Db = int(D[b])
                  off = int(offs[b])
                  S = slice(off, off + Db)
                  s_t = scrp.tile([128, 1], F32, tag="p2s")
                  nc.vector.tensor_reduce(s_t[:], e_t[:, S],
                                          axis=mybir.AxisListType.X, op=ALU.add)
                  nc.vector.tensor_scalar(
                      out=g_t[:, S], in0=e_t[:, S], scalar1=s_t[:], scalar2=None,
                      op0=ALU.subtract)
              omu = gpool.tile([128, SUMD], F32, tag="p2i")
              nc.vector.tensor_sub(omu[:], ones[:], ugrid[:])
              t1 = gpool.tile([128, SUMD], F32, tag="p2j")
              nc.vector.tensor_mul(t1[:], e_t[:], ugrid[:])
              x_t = gpool.tile([128, SUMD], F32, tag="p2k")
              nc.vector.scalar_tensor_tensor(
                  out=x_t[:], in0=g_t[:], scalar=0.0, in1=omu[:],
                  op0=ALU.add, op1=ALU.mult)
              nc.vector.tensor_add(x_t[:], x_t[:], t1[:])
              nc.vector.tensor_single_scalar(ystg[:], x_t[:], 0.0, ALU.is_gt)

            ysti = gpool.tile([128, SUMD], I32, tag="ysti")
            nc.vector.tensor_copy(ysti[:], ystg[:])
            nmg = gpool.tile([128, SUMD], I32, tag="nmg")
            nc.vector.tensor_copy(nmg[:], emaski[:])
            nc.vector.copy_predicated(nmg[:], ysti[:], hierc[:])
            cmask = gpool.tile([128, SUMD], F32, tag="cmask")
            nc.vector.tensor_max(cmask[:], ystg[:], empos[:])
            cwg = gpool.tile([128, SUMD], F32, tag="cwg")
            nc.vector.tensor_mul(cwg[:], cmask[:], scoresg[:])
            nysts = gpool.tile([128, SUMD], F32, tag="nysts")
            nc.vector.scalar_tensor_tensor(
                out=nysts[:], in0=ystg[:], scalar=-1.0, in1=ones[:],
                op0=ALU.mult, op1=ALU.add)
            smask = gpool.tile([128, SUMD], F32, tag="smask")
            nc.vector.tensor_mul(smask[:], nysts[:], emneg1[:])
            swg = gpool.tile([128, SUMD], F32, tag="swg")
            nc.vector.scalar_tensor_tensor(
                out=swg[:], in0=scoresg[:], scalar=-1.0, in1=smask[:],
                op0=ALU.mult, op1=ALU.mult)

            nc.gpsimd.dma_start(out=scores_o[:], in_=scoresg[:])
            nc.gpsimd.dma_start(out=yst_o[:], in_=ystg[:])
            nc.gpsimd.dma_start(out=nm_o[:], in_=nmg[:])
            nc.gpsimd.dma_start(out=cw_o[:], in_=cwg[:])
            nc.gpsimd.dma_start(out=sw_o[:], in_=swg[:])

    nc.compile()
    return nc


def _unpack_outputs(meta, results):
    E = meta["E"]
    scores = np.zeros(E, np.float32)
    yst = np.zeros(E, np.float32)
    nm = np.zeros(E, np.int32)
    cw = np.zeros(E, np.float32)
    sw = np.zeros(E, np.float32)
    for c in range(N_CORES):
        op = meta["origpos"][c]
        m = op >= 0
        idx = op[m]
        r = results[c]
        scores[idx] = r["scores"][m]
        yst[idx] = r["yst"][m]
        nm[idx] = r["nm"][m]
        cw[idx] = r["cw"][m]
        sw[idx] = r["sw"][m]
    return scores, yst, nm, cw, sw


_CACHE = {}


def _get_program(meta, repeats=1):
    key = (meta["NB"], tuple(meta["D"]), meta["PP"], meta["hier1"],
           meta["b2"], meta["N"], repeats)
    if key not in _CACHE:
        _CACHE[key] = _build_program(meta, repeats)
    return _CACHE[key]


def kernel(h, W1, b1, W2, b2, u, row, col, edge_mask, hierarchy):
    h = np.asarray(h, np.float32)
    W1 = np.asarray(W1, np.float32)
    b1 = np.asarray(b1, np.float32)
    W2 = np.asarray(W2, np.float32)
    b2 = np.asarray(b2, np.float32)
    u = np.asarray(u, np.float32)
    row = np.asarray(row, np.int32)
    col = np.asarray(col, np.int32)
    edge_mask = np.asarray(edge_mask, np.int32)
    meta, in_maps = _pack_inputs(h, W1, b1, W2, b2, u, row, col, edge_mask,
                                 int(hierarchy))
    nc = _get_program(meta)
    res = run_bass_kernel_spmd(nc, in_maps, core_ids=list(range(N_CORES)))
    return _unpack_outputs(meta, res.results)



# revision 5
# speedup vs baseline: 3.5359x; 3.5359x over previous
"""TRN2 Bass kernel for nn_DecompModel (GNN edge scorer), 8-core SPMD.

kernel(**inputs) takes the FULL inputs and returns the full 5-tuple
(scores, y_st, new_mask, causal_w, spu_w), computed on 8 NeuronCores.

Strategy (see block comments): edges are partitioned by source node across
cores; per core, nodes are degree-sorted into blocks of 128 partitions with
per-block slot columns, so the segment softmax is a free-dim reduction.
Edge MLP runs on the PE as a single plain fp32 matmul per slot column
(per-instruction cost dominates on this platform, and fp32 matmul is the
cheapest instruction); the W2 contraction is fused into ACT Relu accum
with |w2| folded into W1 (positive/negative column groups). The Bernoulli
threshold uses the log-free  e*u + (e-s)*(1-u) > 0  with polynomial exp.
"""
import numpy as np
import concourse.bass as bass
import concourse.bacc as bacc
import concourse.mybir as mybir
import concourse.tile as tile
from concourse.bass_utils import run_bass_kernel_spmd

F32 = mybir.dt.float32
F32R = mybir.dt.float32r
I32 = mybir.dt.int32
I16 = mybir.dt.int16
ALU = mybir.AluOpType
ACTF = mybir.ActivationFunctionType

N_CORES = 8
EMB = 128
HID = 512

_EXP_C = [1.0, 1.0, 1 / 2, 1 / 6, 1 / 24, 1 / 120, 1 / 720, 1 / 5040]
_LOG2E = 1.4426950408889634
_LN2_HI = np.float32(0.6931471824645996)
_LN2_LO = np.float32(-1.904654323148236e-09)
_MAGIC = np.float32(12582912.0)


def _pack_inputs(h, W1, b1, W2, b2, u, row, col, edge_mask, hierarchy):
    N = h.shape[0]
    E = row.shape[0]
    deg = np.bincount(row, minlength=N)
    cum = np.cumsum(deg)
    bounds = [0]
    for c in range(1, N_CORES):
        bounds.append(int(np.searchsorted(cum, E * c / N_CORES)))
    bounds.append(N)

    order = np.argsort(row, kind="stable")
    starts = np.concatenate([[0], cum])

    maxnodes = max(b - a for a, b in zip(bounds[:-1], bounds[1:]))
    NB = (maxnodes + 127) // 128

    core_nodes = []
    Ds = np.zeros((N_CORES, NB), np.int64)
    for c in range(N_CORES):
        a, b = bounds[c], bounds[c + 1]
        nodes = np.arange(a, b)
        srt = nodes[np.argsort(-deg[a:b], kind="stable")]
        srt = np.concatenate([srt, np.zeros(NB * 128 - len(srt), np.int64)])
        dg = deg[srt].copy()
        dg[b - a:] = 0
        core_nodes.append((srt, dg))
        Ds[c] = dg.reshape(NB, 128).max(1)
    D = np.maximum(Ds.max(0).astype(int), 1)
    SUMD = int(D.sum())
    offs = np.concatenate([[0], np.cumsum(D)]).astype(int)

    aw2 = np.abs(W2[:, 0])
    sgn_neg = W2[:, 0] < 0
    perm = np.argsort(sgn_neg, kind="stable")
    PP = int((~sgn_neg).sum())
    W1s = (W1 * aw2[None, :])[:, perm]
    b1s = (b1 * aw2)[perm]
    assert np.abs(b1s).max() == 0.0, "nonzero b1 not wired up"
    Wtop = np.ascontiguousarray(W1s[:EMB])
    Wbot = np.ascontiguousarray(W1s[EMB:])

    def wrap16(idxs):
        n = len(idxs)
        w = np.zeros((16, n // 16), np.int16)
        ii = np.arange(n)
        w[ii % 16, ii // 16] = idxs
        return np.tile(w, (8, 1))

    in_maps = []
    origs = []
    for c in range(N_CORES):
        srt, dg = core_nodes[c]
        colgrid = np.zeros((128, SUMD), np.int64)
        ugrid = np.full((128, SUMD), 0.5, np.float32)
        valid = np.zeros((128, SUMD), np.float32)
        emi = np.full((128, SUMD), -1, np.int32)
        origpos = np.full((128, SUMD), -1, np.int64)
        for b in range(NB):
            for p in range(128):
                node = srt[b * 128 + p]
                d = dg[b * 128 + p]
                if d == 0:
                    continue
                eids = order[starts[node]:starts[node] + d]
                sl = slice(offs[b], offs[b] + d)
                colgrid[p, sl] = col[eids]
                ugrid[p, sl] = u[eids]
                valid[p, sl] = 1.0
                emi[p, sl] = edge_mask[eids]
                origpos[p, sl] = eids
        nodeidx = wrap16(srt.astype(np.int16))
        colw = np.concatenate(
            [wrap16(colgrid[:, offs[b]:offs[b + 1]].T.ravel().astype(np.int16))
             for b in range(NB)], axis=1)
        in_maps.append({
            "h": np.ascontiguousarray(h),
            "wtop": Wtop, "wbot": Wbot,
            "nodeidx": nodeidx, "colidx": colw,
            "ugrid": ugrid, "valid": valid,
            "emaski": emi,
            "empos": (emi > 0).astype(np.float32),
            "emneg1": (emi == -1).astype(np.float32),
            "ident": np.eye(128, dtype=np.float32),
        })
        origs.append(origpos)
    meta = dict(NB=NB, D=D, SUMD=SUMD, offs=offs, PP=PP, origpos=origs,
                E=E, N=N, hier1=int(hierarchy) + 1, b2=float(np.ravel(b2)[0]))
    return meta, in_maps


def _build_program(meta, repeats=1):
    NB, D, SUMD, offs, PP = (meta["NB"], meta["D"], meta["SUMD"],
                             meta["offs"], meta["PP"])
    N = meta["N"]
    nc = bacc.Bacc("TRN2", target_bir_lowering=False, debug=False,
                   num_devices=N_CORES)
    dt_ = nc.dram_tensor
    h_d = dt_("h", [N, EMB], F32, kind="ExternalInput").ap()
    wtop_d = dt_("wtop", [EMB, HID], F32, kind="ExternalInput").ap()
    wbot_d = dt_("wbot", [EMB, HID], F32, kind="ExternalInput").ap()
    nodeidx_d = dt_("nodeidx", [128, NB * 8], I16, kind="ExternalInput").ap()
    colidx_d = dt_("colidx", [128, 8 * SUMD], I16, kind="ExternalInput").ap()
    ugrid_d = dt_("ugrid", [128, SUMD], F32, kind="ExternalInput").ap()
    valid_d = dt_("valid", [128, SUMD], F32, kind="ExternalInput").ap()
    emaski_d = dt_("emaski", [128, SUMD], I32, kind="ExternalInput").ap()
    empos_d = dt_("empos", [128, SUMD], F32, kind="ExternalInput").ap()
    emneg1_d = dt_("emneg1", [128, SUMD], F32, kind="ExternalInput").ap()
    ident_d = dt_("ident", [128, 128], F32, kind="ExternalInput").ap()

    scores_o = dt_("scores", [128, SUMD], F32, kind="ExternalOutput").ap()
    yst_o = dt_("yst", [128, SUMD], F32, kind="ExternalOutput").ap()
    nm_o = dt_("nm", [128, SUMD], I32, kind="ExternalOutput").ap()
    cw_o = dt_("cw", [128, SUMD], F32, kind="ExternalOutput").ap()
    sw_o = dt_("sw", [128, SUMD], F32, kind="ExternalOutput").ap()

    with tile.TileContext(nc) as tc:
        with (
            tc.tile_pool(name="const", bufs=1) as cpool,
            tc.tile_pool(name="grids", bufs=1) as gpool,
            tc.tile_pool(name="gath", bufs=3) as gath,
            tc.tile_pool(name="work", bufs=10) as work,
            tc.tile_pool(name="scr", bufs=8) as scrp,
            tc.tile_pool(name="ps_tr", bufs=2, space="PSUM") as ps_tr,
            tc.tile_pool(name="ps_hid", bufs=6, space="PSUM") as ps_hid,
        ):
            wtop_f = cpool.tile([EMB, HID], F32, tag="wtopf")
            nc.gpsimd.dma_start(out=wtop_f[:], in_=wtop_d[:])
            wbot_f = cpool.tile([EMB, HID], F32, tag="wbotf")
            nc.gpsimd.dma_start(out=wbot_f[:], in_=wbot_d[:])
            ident = cpool.tile([128, 128], F32, tag="ident")
            nc.gpsimd.dma_start(out=ident[:], in_=ident_d[:])

            nodeidx = cpool.tile([128, NB * 8], I16, tag="nodeidx")
            nc.gpsimd.dma_start(out=nodeidx[:], in_=nodeidx_d[:])
            colidx = cpool.tile([128, 8 * SUMD], I16, tag="colidx")
            nc.gpsimd.dma_start(out=colidx[:], in_=colidx_d[:])
            ugrid = gpool.tile([128, SUMD], F32, tag="ugrid")
            nc.gpsimd.dma_start(out=ugrid[:], in_=ugrid_d[:])
            validg = gpool.tile([128, SUMD], F32, tag="valid")
            nc.gpsimd.dma_start(out=validg[:], in_=valid_d[:])
            emaski = gpool.tile([128, SUMD], I32, tag="emaski")
            nc.gpsimd.dma_start(out=emaski[:], in_=emaski_d[:])
            empos = gpool.tile([128, SUMD], F32, tag="empos")
            nc.gpsimd.dma_start(out=empos[:], in_=empos_d[:])
            emneg1 = gpool.tile([128, SUMD], F32, tag="emneg1")
            nc.gpsimd.dma_start(out=emneg1[:], in_=emneg1_d[:])

            ones = gpool.tile([128, SUMD], F32, tag="ones")
            nc.vector.memset(ones[:], 1.0)
            hierc = gpool.tile([128, SUMD], I32, tag="hierc")
            nc.vector.memset(hierc[:], meta["hier1"])

            accp = gpool.tile([128, SUMD], F32, tag="accp")
            accn = gpool.tile([128, SUMD], F32, tag="accn")
            scoresg = gpool.tile([128, SUMD], F32, tag="scoresg")
            ystg = gpool.tile([128, SUMD], F32, tag="ystg")

            for rep in range(repeats):
              for b in range(NB):
                  Db = int(D[b])
                  off = int(offs[b])
                  hrow_g = gath.tile([128, 1, 128], F32, tag="hrowg")
                  nc.gpsimd.dma_gather(
                      out_ap=hrow_g[:],
                      in_ap=h_d[:], idxs_ap=nodeidx[:, b * 8:(b + 1) * 8],
                      num_idxs=128, num_idxs_reg=128, elem_size=EMB)
                  trp = ps_tr.tile([128, 128], F32, tag="tr")
                  nc.tensor.transpose(trp[:], hrow_g[:, 0, :], ident[:])
                  hr1 = work.tile([128, 128], F32, tag="hr1")
                  nc.vector.tensor_copy(hr1[:], trp[:])
                  a_ps = ps_hid.tile([128, HID], F32, tag="hid")
                  nc.tensor.matmul(a_ps[:], hr1[:], wtop_f[:], start=True, stop=True)
                  a_sb = work.tile([128, HID], F32, tag="asb")
                  nc.vector.tensor_copy(a_sb[:], a_ps[:])

                  hcol_g = gath.tile([128, Db, 128], F32, tag="hcolg")
                  CH = 8  # slot-columns per dma_gather (1024 idxs)
                  for j0 in range(0, Db, CH):
                      jn = min(CH, Db - j0)
                      nc.gpsimd.dma_gather(
                          out_ap=hcol_g[:, j0:j0 + jn, :],
                          in_ap=h_d[:],
                          idxs_ap=colidx[:, 8 * (off + j0):8 * (off + j0 + jn)],
                          num_idxs=128 * jn, num_idxs_reg=128 * jn,
                          elem_size=EMB)

                  PC = 8  # phase chunk
                  for j0 in range(0, Db, PC):
                      jn = min(PC, Db - j0)
                      ngrp = (jn + 3) // 4
                      trcs = []
                      for g in range(ngrp):
                          trc = ps_tr.tile([128, 512], F32, tag="tr")
                          trcs.append(trc)
                      for t in range(jn):
                          nc.tensor.transpose(
                              trcs[t // 4][:, (t % 4) * 128:(t % 4 + 1) * 128],
                              hcol_g[:, j0 + t, :], ident[:])
                      hcs = []
                      for g in range(ngrp):
                          hc = work.tile([128, 512], F32, tag="hc")
                          nc.vector.tensor_copy(hc[:], trcs[g][:])
                          hcs.append(hc)
                      hid_pss = []
                      for t in range(jn):
                          hid_ps = ps_hid.tile([128, HID], F32, tag="hid")
                          nc.tensor.matmul(
                              hid_ps[:],
                              hcs[t // 4][:, (t % 4) * 128:(t % 4 + 1) * 128],
                              wbot_f[:], start=True, stop=True)
                          hid_pss.append(hid_ps)
                      for t in range(jn):
                          jj = off + j0 + t
                          hidf = work.tile([128, HID], F32, tag="hidf")
                          nc.vector.tensor_add(hidf[:], hid_pss[t][:], a_sb[:])
                          scr = scrp.tile([128, HID], F32, tag="scr")
                          nc.scalar.activation(scr[:, :PP], hidf[:, :PP], ACTF.Relu,
                                               accum_out=accp[:, jj:jj + 1])
                          nc.scalar.activation(scr[:, PP:], hidf[:, PP:], ACTF.Relu,
                                               accum_out=accn[:, jj:jj + 1])

              # ---- global post chain over [128, SUMD] ----
              sc = scoresg[:]
              nc.vector.tensor_sub(sc, accp[:], accn[:])
              if meta["b2"] != 0.0:
                  nc.vector.tensor_scalar_add(sc, sc, meta["b2"])
              t_kf = gpool.tile([128, SUMD], F32, tag="p2a")
              nc.vector.tensor_scalar_mul(t_kf[:], sc, float(_LOG2E))
              t_m = gpool.tile([128, SUMD], F32, tag="p2b")
              nc.vector.tensor_scalar_add(t_m[:], t_kf[:], float(_MAGIC))
              t_kr = gpool.tile([128, SUMD], F32, tag="p2c")
              nc.vector.tensor_scalar_sub(t_kr[:], t_m[:], float(_MAGIC))
              t_f = gpool.tile([128, SUMD], F32, tag="p2d")
              nc.vector.scalar_tensor_tensor(
                  out=t_f[:], in0=t_kr[:], scalar=-float(_LN2_HI),
                  in1=sc, op0=ALU.mult, op1=ALU.add)
              nc.vector.scalar_tensor_tensor(
                  out=t_f[:], in0=t_kr[:], scalar=-float(_LN2_LO),
                  in1=t_f[:], op0=ALU.mult, op1=ALU.add)
              t_acc = gpool.tile([128, SUMD], F32, tag="p2e")
              nc.vector.memset(t_acc[:], _EXP_C[-1])
              for kcoef in reversed(_EXP_C[:-1]):
                  nc.vector.scalar_tensor_tensor(
                      out=t_acc[:], in0=t_acc[:], scalar=0.0,
                      in1=t_f[:], op0=ALU.add, op1=ALU.mult)
                  nc.vector.tensor_scalar_add(t_acc[:], t_acc[:], float(kcoef))
              t_mi = gpool.tile([128, SUMD], I32, tag="p2l")
              nc.vector.tensor_copy(t_mi[:], t_m[:])
              t_bits = gpool.tile([128, SUMD], I32, tag="p2f")
              nc.vector.tensor_scalar(
                  out=t_bits[:], in0=t_mi[:],
                  scalar1=(127 - 12582912), scalar2=(1 << 23),
                  op0=ALU.add, op1=ALU.mult)
              e_t = gpool.tile([128, SUMD], F32, tag="p2g")
              nc.vector.tensor_mul(e_t[:], t_acc[:], t_bits[:].bitcast(F32))
              nc.vector.tensor_mul(e_t[:], e_t[:], validg[:])
              g_t = gpool.tile([128, SUMD], F32, tag="p2h")
              for b in range(NB):
                  Db = int(D[b])
                  off = int(offs[b])
                  S = slice(off, off + Db)
                  s_t = scrp.tile([128, 1], F32, tag="p2s")
                  nc.vector.tensor_reduce(s_t[:], e_t[:, S],
                                          axis=mybir.AxisListType.X, op=ALU.add)
                  nc.vector.tensor_scalar(
                      out=g_t[:, S], in0=e_t[:, S], scalar1=s_t[:], scalar2=None,
                      op0=ALU.subtract)
              omu = gpool.tile([128, SUMD], F32, tag="p2i")
              nc.vector.tensor_sub(omu[:], ones[:], ugrid[:])
              t1 = gpool.tile([128, SUMD], F32, tag="p2j")
              nc.vector.tensor_mul(t1[:], e_t[:], ugrid[:])
              x_t = gpool.tile([128, SUMD], F32, tag="p2k")
              nc.vector.scalar_tensor_tensor(
                  out=x_t[:], in0=g_t[:], scalar=0.0, in1=omu[:],
                  op0=ALU.add, op1=ALU.mult)
              nc.vector.tensor_add(x_t[:], x_t[:], t1[:])
              nc.vector.tensor_single_scalar(ystg[:], x_t[:], 0.0, ALU.is_gt)

            ysti = gpool.tile([128, SUMD], I32, tag="ysti")
            nc.vector.tensor_copy(ysti[:], ystg[:])
            nmg = gpool.tile([128, SUMD], I32, tag="nmg")
            nc.vector.tensor_copy(nmg[:], emaski[:])
            nc.vector.copy_predicated(nmg[:], ysti[:], hierc[:])
            cmask = gpool.tile([128, SUMD], F32, tag="cmask")
            nc.vector.tensor_max(cmask[:], ystg[:], empos[:])
            cwg = gpool.tile([128, SUMD], F32, tag="cwg")
            nc.vector.tensor_mul(cwg[:], cmask[:], scoresg[:])
            nysts = gpool.tile([128, SUMD], F32, tag="nysts")
            nc.vector.scalar_tensor_tensor(
                out=nysts[:], in0=ystg[:], scalar=-1.0, in1=ones[:],
                op0=ALU.mult, op1=ALU.add)
            smask = gpool.tile([128, SUMD], F32, tag="smask")
            nc.vector.tensor_mul(smask[:], nysts[:], emneg1[:])
            swg = gpool.tile([128, SUMD], F32, tag="swg")
            nc.vector.scalar_tensor_tensor(
                out=swg[:], in0=scoresg[:], scalar=-1.0, in1=smask[:],
                op0=ALU.mult, op1=ALU.mult)

            nc.gpsimd.dma_start(out=scores_o[:], in_=scoresg[:])
            nc.gpsimd.dma_start(out=yst_o[:], in_=ystg[:])
            nc.gpsimd.dma_start(out=nm_o[:], in_=nmg[:])
            nc.gpsimd.dma_start(out=cw_o[:], in_=cwg[:])
            nc.gpsimd.dma_start(out=sw_o[:], in_=swg[:])

    nc.compile()
    return nc


def _unpack_outputs(meta, results):
    E = meta["E"]
    scores = np.zeros(E, np.float32)
    yst = np.zeros(E, np.float32)
    nm = np.zeros(E, np.int32)
    cw = np.zeros(E, np.float32)
    sw = np.zeros(E, np.float32)
    for c in range(N_CORES):
        op = meta["origpos"][c]
        m = op >= 0
        idx = op[m]
        r = results[c]
        scores[idx] = r["scores"][m]
        yst[idx] = r["yst"][m]
        nm[idx] = r["nm"][m]
        cw[idx] = r["cw"][m]
        sw[idx] = r["sw"][m]
    return scores, yst, nm, cw, sw


_CACHE = {}


def _get_program(meta, repeats=1):
    key = (meta["NB"], tuple(meta["D"]), meta["PP"], meta["hier1"],
           meta["b2"], meta["N"], repeats)
    if key not in _CACHE:
        _CACHE[key] = _build_program(meta, repeats)
    return _CACHE[key]


def kernel(h, W1, b1, W2, b2, u, row, col, edge_mask, hierarchy):
    h = np.asarray(h, np.float32)
    W1 = np.asarray(W1, np.float32)
    b1 = np.asarray(b1, np.float32)
    W2 = np.asarray(W2, np.float32)
    b2 = np.asarray(b2, np.float32)
    u = np.asarray(u, np.float32)
    row = np.asarray(row, np.int32)
    col = np.asarray(col, np.int32)
    edge_mask = np.asarray(edge_mask, np.int32)
    meta, in_maps = _pack_inputs(h, W1, b1, W2, b2, u, row, col, edge_mask,
                                 int(hierarchy))
    nc = _get_program(meta)
    res = run_bass_kernel_spmd(nc, in_maps, core_ids=list(range(N_CORES)))
    return _unpack_outputs(meta, res.results)

